# revision 29
# baseline (speedup 1.0000x reference)
"""Trainium2 Bass kernel for nn_BatchedTeacherPolicy.

2048 independent per-teacher MLPs (obs-norm -> 48->512->256->128->12,
ELU between layers, tanh at the end). Pure data parallel: 256 teachers
per NeuronCore across 8 cores.

Layout: teacher-on-partition. Each SBUF partition holds one teacher's
weights/activations; the per-teacher matvec y[o] = b[o] + sum_i W[o,i]x[i]
is one fused DVE tensor_tensor_reduce per output neuron o, computed for
128 teachers (partitions) simultaneously. Weight DMAs are fully
contiguous per partition (W[n, o0:o1, :] blocks).
"""

from contextlib import ExitStack

import numpy as np

import concourse.bass as bass
import concourse.bacc as bacc
import concourse.tile as tile
from concourse import mybir
from concourse.bass_utils import run_bass_kernel_spmd

N, OBS = 2048, 48
DIMS = [(512, 48), (256, 512), (128, 256), (12, 128)]  # (out, in) per layer
N_CORES = 8
NPC = N // N_CORES  # teachers per core
P = 128             # partitions = teachers per group
G = NPC // P        # groups per core
# o-chunk per layer: sized so W DMA chunks are ~2-4 MB
OCHUNK = [128, 12, 16, 12]

F32 = mybir.dt.float32
AF = mybir.ActivationFunctionType
ALU = mybir.AluOpType

# Layer 1 output split: o < L1_DVE computed by DVE fused multiply-reduce;
# the last L1_PE columns computed on TensorE from a host-transposed W1
# slice (keeps DVE, the bottleneck engine, under the DMA roofline).
L1_PE = 64
L1_DVE = DIMS[1][0] - L1_PE
L1_CI = DIMS[1][1] // P  # 4 contraction chunks of 128

_cached = {}


def _build_bass():
    nc = bacc.Bacc(trn_type="TRN2", target_bir_lowering=False)

    obs_d = nc.dram_tensor("obs", [NPC, OBS], F32, kind="ExternalInput")
    mean_d = nc.dram_tensor("mean", [NPC, OBS], F32, kind="ExternalInput")
    std_d = nc.dram_tensor("std", [NPC, OBS], F32, kind="ExternalInput")
    W_d, b_d = [], []
    for li, (o, i) in enumerate(DIMS):
        o_dve = L1_DVE if li == 1 else o
        W_d.append(
            nc.dram_tensor(f"W{li}", [NPC, o_dve, i], F32, kind="ExternalInput")
        )
        b_d.append(nc.dram_tensor(f"b{li}", [NPC, o], F32, kind="ExternalInput"))
    # host-transposed slice of W1: [g, ci, i_local(part), teacher, o]
    w1t_d = nc.dram_tensor(
        "W1T", [G, L1_CI, P, P, L1_PE], F32, kind="ExternalInput"
    )
    out_d = nc.dram_tensor("out", [NPC, DIMS[-1][0]], F32, kind="ExternalOutput")

    from concourse.masks import make_identity

    with ExitStack() as ctx:
        tc = ctx.enter_context(tile.TileContext(nc))
        wpool = ctx.enter_context(tc.tile_pool(name="wpool", bufs=4))
        xpool = ctx.enter_context(tc.tile_pool(name="xpool", bufs=4))
        spool = ctx.enter_context(tc.tile_pool(name="spool", bufs=2))
        bpool = ctx.enter_context(tc.tile_pool(name="bpool", bufs=2))
        ppool = ctx.enter_context(tc.tile_pool(name="ppool", bufs=2, space="PSUM"))
        ipool = ctx.enter_context(tc.tile_pool(name="ipool", bufs=1))

        ident = ipool.tile([P, P], F32)
        make_identity(nc, ident)

        def emit_norm(g):
            n0 = g * P

            # ---- obs normalization: x0 = clip((obs - mean)/std, -5, 5) ----
            obs_t = spool.tile([P, OBS], F32, tag="nrm")
            nc.sync.dma_start(out=obs_t, in_=obs_d[n0 : n0 + P, :])
            mean_t = spool.tile([P, OBS], F32, tag="nrm")
            nc.sync.dma_start(out=mean_t, in_=mean_d[n0 : n0 + P, :])
            std_t = spool.tile([P, OBS], F32, tag="nrm")
            nc.sync.dma_start(out=std_t, in_=std_d[n0 : n0 + P, :])

            # Each DVE op may carry at most ONE new semaphore wait (TRN2
            # TT-struct limit), so feed multi-operand ops through
            # single-input ops that absorb the DMA waits first.
            nmean = spool.tile([P, OBS], F32, tag="nmean")
            nc.vector.tensor_scalar_mul(nmean, mean_t, -1.0)
            rstd = spool.tile([P, OBS], F32, tag="rstd")
            nc.vector.reciprocal(rstd, std_t)
            x = xpool.tile([P, OBS], F32, tag="x", name=f"x_in_{g}")
            nc.vector.tensor_add(x, obs_t, nmean)
            nc.vector.tensor_mul(x, x, rstd)
            nc.vector.tensor_scalar(
                out=x, in0=x, scalar1=-5.0, scalar2=5.0,
                op0=ALU.max, op1=ALU.min,
            )
            return x

        def emit_layer(g, li, x):
            n0 = g * P
            O, I = DIMS[li]
            if True:
                bt = bpool.tile([P, O], F32, tag="bias", name=f"b_{g}_{li}")
                nc.sync.dma_start(out=bt, in_=b_d[li][n0 : n0 + P, :])
                y = xpool.tile([P, O], F32, tag="y", name=f"y_{g}_{li}")
                o_dve = L1_DVE if li == 1 else O

                if li == 1:
                    # TensorE path for y[:, L1_DVE:]: x1 transposed via PE,
                    # then per-teacher matvecs with the host-transposed W1
                    # slice as the stationary operand. ci-outer order keeps
                    # exactly one W1T tile live at a time; each PSUM column
                    # t accumulates across the four ci passes.
                    x1t = xpool.tile([P, L1_CI, P], F32, tag="x1t", name=f"x1t_{g}")
                    for ci in range(L1_CI):
                        pst = ppool.tile([P, P], F32, tag="pst", name=f"pst_{g}_{ci}")
                        nc.tensor.transpose(
                            pst, x[:, ci * P : (ci + 1) * P], ident
                        )
                        nc.scalar.copy(x1t[:, ci, :], pst)
                    yps = ppool.tile([L1_PE, P], F32, tag="yps", name=f"yps_{g}")
                    TH = 32  # teachers per W1T DMA tile
                    for th0 in range(0, P, TH):
                        wtts = []
                        for ci in range(L1_CI):
                            wtt = wpool.tile(
                                [P, TH, L1_PE], F32, tag="w1t", bufs=6,
                                name=f"w1t_{g}_{th0}_{ci}",
                            )
                            # SWDGE (gpsimd) path: these DMAs wait on PE
                            # slot reuse; on the SP or ACT HWDGE rings that
                            # wait would block the weight stream / the ELU
                            # Exp ops queued behind them (HWDGE is FIFO per
                            # issuing engine).
                            nc.gpsimd.dma_start(
                                out=wtt, in_=w1t_d[g, ci, :, th0 : th0 + TH, :]
                            )
                            wtts.append(wtt)
                        # t-outer, ci-inner: each PSUM column's accumulation
                        # group runs start..stop contiguously (interleaved
                        # groups lose earlier columns' start contributions).
                        for tl in range(TH):
                            t = th0 + tl
                            for ci in range(L1_CI):
                                nc.tensor.matmul(
                                    yps[:, t : t + 1],
                                    lhsT=wtts[ci][:, tl, :],
                                    rhs=x1t[:, ci, t : t + 1],
                                    start=(ci == 0),
                                    stop=(ci == L1_CI - 1),
                                )
                    m1 = xpool.tile([L1_PE, P], F32, tag="m1", name=f"m1_{g}")
                    nc.scalar.copy(m1, yps)
                    pst2 = ppool.tile([P, L1_PE], F32, tag="pst2", name=f"pst2_{g}")
                    nc.tensor.transpose(pst2, m1, ident[:L1_PE, :L1_PE])
                    nc.scalar.copy(y[:, L1_DVE:O], pst2)

                for c0 in range(0, o_dve, OCHUNK[li]):
                    oc = min(OCHUNK[li], o_dve - c0)
                    wt = wpool.tile([P, oc, I], F32, tag="w", name=f"w_{g}_{li}_{c0}")
                    nc.sync.dma_start(
                        out=wt, in_=W_d[li][n0 : n0 + P, c0 : c0 + oc, :]
                    )
                    if I <= 64:
                        # Layer 0: I is tiny, so per-o fused ops are
                        # overhead-dominated. Instead: one in-place batched
                        # multiply (x broadcast across the o dim via a
                        # step-0 AP) + one segmented 3D reduce.
                        x_b = bass.AP(
                            tensor=x.tensor,
                            offset=x.offset,
                            ap=[x.ap[0], [0, oc], x.ap[1]],
                        )
                        nc.vector.tensor_mul(wt, wt, x_b)
                        nc.vector.reduce_sum(
                            out=y[:, c0 : c0 + oc],
                            in_=wt,
                            axis=mybir.AxisListType.X,
                        )
                    else:
                        scr = spool.tile(
                            [P, I], F32, tag="scr", name=f"scr_{g}_{li}_{c0}"
                        )
                        for o in range(oc):
                            # accum_out = sum_i W[o,i]*x[i]  (custom DVE
                            # fused multiply-reduce; the ISA
                            # TENSOR_TENSOR_REDUCE opcode crashes TRN2
                            # hardware on this path)
                            nc.vector.affine_mul_reduce(
                                out=scr,
                                accum_out=y[:, c0 + o : c0 + o + 1],
                                in0=wt[:, o, :],
                                in1=x,
                                scale=1.0,
                                bias=0.0,
                            )
                nc.vector.tensor_add(y, y, bt)
                if li < len(DIMS) - 1:
                    # ELU(y) = exp(min(y,0)) + max(y,0) - 1
                    e = spool.tile([P, O], F32, tag="elu", name=f"e_{g}_{li}")
                    nc.vector.tensor_scalar_min(e, y, 0.0)
                    nc.scalar.activation(e, e, AF.Exp)
                    xn = xpool.tile([P, O], F32, tag="x", name=f"x_{g}_{li}")
                    nc.vector.scalar_tensor_tensor(
                        out=xn, in0=y, scalar=0.0, in1=e,
                        op0=ALU.max, op1=ALU.add,
                    )
                    nc.vector.tensor_scalar_add(xn, xn, -1.0)
                    return xn
                yt = xpool.tile([P, O], F32, tag="yt", name=f"yt_{g}")
                nc.scalar.activation(yt, y, AF.Tanh)
                nc.scalar.dma_start(out=out_d[n0 : n0 + P, :], in_=yt)
                return None

        # Staggered two-group pipeline: group 1 runs one layer behind group
        # 0 so DVE-heavy L0 work overlaps the other group's DMA-heavy L1
        # phase (and the PE matvec phase always has DVE work available).
        x0 = emit_norm(0)
        x0 = emit_layer(0, 0, x0)
        x1 = emit_norm(1)
        x0 = emit_layer(0, 1, x0)
        x1 = emit_layer(1, 0, x1)
        x1 = emit_layer(1, 1, x1)
        x0 = emit_layer(0, 2, x0)
        emit_layer(0, 3, x0)
        x1 = emit_layer(1, 2, x1)
        emit_layer(1, 3, x1)

    nc.compile()
    return nc


def _get_nc():
    if "nc" not in _cached:
        _cached["nc"] = _build_bass()
    return _cached["nc"]


def _pack_core_inputs(full, c):
    """Shard + lay out one core's inputs (including the transposed W1 slice)."""
    sl = slice(c * NPC, (c + 1) * NPC)
    m = {
        k: np.ascontiguousarray(np.asarray(v)[sl])
        for k, v in full.items()
        if k != "W1"
    }
    w1c = np.asarray(full["W1"])[sl]  # [NPC, 256, 512]
    m["W1"] = np.ascontiguousarray(w1c[:, :L1_DVE, :])
    w1b = w1c[:, L1_DVE:, :]  # [NPC, L1_PE, 512]
    # -> [g, ci, i_local, teacher, o]
    m["W1T"] = np.ascontiguousarray(
        w1b.reshape(G, P, L1_PE, L1_CI, P).transpose(0, 3, 4, 1, 2)
    )
    return m


def kernel(obs, mean, std, W0, b0, W1, b1, W2, b2, W3, b3, _trace=False):
    nc = _get_nc()
    full = {
        "obs": obs, "mean": mean, "std": std,
        "W0": W0, "b0": b0, "W1": W1, "b1": b1,
        "W2": W2, "b2": b2, "W3": W3, "b3": b3,
    }
    in_maps = [_pack_core_inputs(full, c) for c in range(N_CORES)]
    res = run_bass_kernel_spmd(
        nc, in_maps, core_ids=list(range(N_CORES)), trace=_trace
    )
    _cached["last_results"] = res
    out = np.concatenate([res.results[c]["out"] for c in range(N_CORES)], axis=0)
    return out
